# revision 1
# baseline (speedup 1.0000x reference)
"""Trainium2 Bass kernel for nn_Encoder_Flows (3-layer dense GCN message passing).

Math per graph (reference):
    A = flows [N, N];  deg[c] = sum_r A[r, c];  dinv = rsqrt(deg)
    L(x, W, b) = dinv * (A^T @ (dinv * (x @ W))) + b
    out = L(L(L(A, W1, b1), W2, b2), W3, b3)          # [N, 128]

Strategy: data-parallel over the batch (16 graphs / 8 cores = 2 graphs per
core, processed sequentially inside one NEFF). A is cast to bf16 on the host
and kept resident in SBUF (double-buffered across graphs). The layer-1
feature matmul A @ W1 needs A-transposed tiles; those come from hardware
DMA-transpose (bf16-only feature) streamed per 128-column strip. All other
stages pick matmul operand roles so that no on-chip transpose is ever
needed:
  u1   = A @ W1          : lhsT = W1[fb], rhs = At strips (DMA-transposed)
                           -> feat-major, then 16 PE transposes for msg1
  y1   = A^T @ msg1      : lhsT = msg1, rhs = A (N=512 chunks)           -> feat-major
  v2   = y1 @ W2         : lhsT = y1T tiles (feat-major is the lhsT!)    -> node-major
  y2   = A^T @ msg2      : like y1, two 128-col halves                   -> feat-major
  v3   = y2 @ W3         : lhsT = y2T tiles                              -> node-major
  y3   = A^T @ msg3      : lhsT = A tiles (stationary), rhs = msg3       -> node-major
deg comes free as a DVE free-axis reduction over the DMA-transposed strips.
All matmuls accumulate in fp32 PSUM; dinv/scalings in fp32.
"""

import sys
from contextlib import ExitStack

import numpy as np

for _p in ("/opt/trn_rl_repo", "/opt/pypackages"):
    if _p not in sys.path:
        sys.path.append(_p)

import ml_dtypes

B, N, P = 16, 2048, 128
NB = N // P          # 16 row/col blocks
NCORES = 8
GPC = B // NCORES    # graphs per core
D1, D2, D3 = 128, 256, 128
CH = 512             # moving-operand chunk
NCH = N // CH

_COMPILED = {}


def _build(with_bias):
    import concourse.mybir as mybir
    import concourse.tile as tile
    from concourse import bacc

    f32 = mybir.dt.float32
    bf16 = mybir.dt.bfloat16

    nc = bacc.Bacc("TRN2", target_bir_lowering=False)
    Ab_d = nc.declare_dram_parameter("Ab", [GPC, N, N], bf16, isOutput=False)
    Abt_d = nc.declare_dram_parameter("Abt", [GPC, N, N], bf16, isOutput=False)
    W1_d = nc.declare_dram_parameter("W1b", [N, D1], bf16, isOutput=False)
    W2_d = nc.declare_dram_parameter("W2b", [D1, D2], bf16, isOutput=False)
    W3_d = nc.declare_dram_parameter("W3b", [D2, D3], bf16, isOutput=False)
    if with_bias:
        c2_d = nc.declare_dram_parameter("c2r", [P, D2], f32, isOutput=False)
        c3_d = nc.declare_dram_parameter("c3r", [P, D3], f32, isOutput=False)
        b3_d = nc.declare_dram_parameter("b3r", [P, D3], f32, isOutput=False)
    out_d = nc.declare_dram_parameter("out", [GPC, N, D3], f32, isOutput=True)

    with tile.TileContext(nc) as tc, ExitStack() as ctx:
        X = mybir.AxisListType.X
        wpool = ctx.enter_context(tc.tile_pool(name="wpool", bufs=1))
        spool = ctx.enter_context(tc.tile_pool(name="spool", bufs=2))
        apool = ctx.enter_context(tc.tile_pool(name="apool", bufs=2))
        strips = ctx.enter_context(tc.tile_pool(name="strips", bufs=6))
        mpool = ctx.enter_context(tc.tile_pool(name="mpool", bufs=1))
        y2pool = ctx.enter_context(tc.tile_pool(name="y2pool", bufs=1))
        psum = ctx.enter_context(tc.tile_pool(name="psum", bufs=1, space="PSUM"))
        psh = ctx.enter_context(tc.tile_pool(name="psh", bufs=4, space="PSUM"))

        # --- weights, replicated constants ---
        W1_sb = wpool.tile([P, NB, D1], bf16)
        nc.sync.dma_start(W1_sb[:], W1_d.ap().rearrange("(fb p) d -> p fb d", p=P))
        W2_sb = wpool.tile([P, D2], bf16)
        nc.sync.dma_start(W2_sb[:], W2_d.ap())
        W3_sb = wpool.tile([P, 2, D3], bf16)
        nc.sync.dma_start(W3_sb[:], W3_d.ap().rearrange("(h p) g -> p h g", p=P))
        if with_bias:
            c2_sb = wpool.tile([P, D2], f32)
            nc.sync.dma_start(c2_sb[:], c2_d.ap())
            c3_sb = wpool.tile([P, D3], f32)
            nc.sync.dma_start(c3_sb[:], c3_d.ap())
            b3_sb = wpool.tile([P, D3], f32)
            nc.sync.dma_start(b3_sb[:], b3_d.ap())

        iob = wpool.tile([P, P], bf16)
        from concourse.masks import make_identity
        make_identity(nc, iob[:])

        out_ap = out_d.ap().rearrange("g (cb p) d -> g p cb d", p=P)

        for g in range(GPC):
            # A split into 4 column-chunk tiles: consumers of chunk ch only
            # wait on chunk ch's DMA, so y1 can start before A fully lands
            A_t = [apool.tile([P, NB, CH], bf16, tag=f"A{q}", name=f"Ac{q}") for q in range(NCH)]

            deg = spool.tile([P, NB], f32, tag="deg")
            dinv = spool.tile([P, NB], f32, tag="dinv")
            rdeg = spool.tile([P, NB], f32, tag="rdeg")

            # ---------- u1T = (A @ W1)^T via DMA-transposed strips; deg free --
            # u1T[d, m] accumulates over fb: lhsT = W1[fb], rhs = At-strip chunks
            u1t = psum.tile([P, N], f32, tag="big")
            for fb in range(NB):
                strip = strips.tile([P, N], bf16, tag="strip")
                nc.sync.dma_start(strip[:], Abt_d.ap()[g][fb * P:(fb + 1) * P, :])
                if fb % 4 == 3:
                    # A natural load interleaved in 512-column chunks: y1's
                    # chunk-major consumption only needs matching columns
                    q = fb // 4
                    nc.sync.dma_start(
                        A_t[q][:],
                        Ab_d.ap()[g].rearrange("(rb p) c -> p rb c", p=P)[:, :, q * CH:(q + 1) * CH])
                nc.vector.reduce_sum(deg[:, fb:fb + 1], strip[:], axis=X)
                for ch in range(NCH):
                    nc.tensor.matmul(
                        u1t[:, ch * CH:(ch + 1) * CH], W1_sb[:, fb, :],
                        strip[:, ch * CH:(ch + 1) * CH],
                        start=(fb == 0), stop=(fb == NB - 1))

            # dinv = sqrt(1/deg); rdeg = 1/deg = dinv^2
            nc.vector.reciprocal(rdeg[:], deg[:])
            nc.scalar.sqrt(dinv[:], rdeg[:])

            # ---------- msg1 = dinv * u1 (node-major via 16 PE transposes) ----
            msg1 = mpool.tile([P, NB, D1], bf16, tag="msg")
            for q in range(4):
                u1q = spool.tile([P, CH], bf16, tag="u1q")
                nc.vector.tensor_copy(u1q[:], u1t[:, q * CH:(q + 1) * CH])
                pt = psh.tile([P, 4, P], bf16, tag="sh")
                for j in range(4):
                    nc.tensor.transpose(pt[:, j, :], u1q[:, j * P:(j + 1) * P], iob[:])
                sl = slice(q * 4, (q + 1) * 4)
                nc.vector.tensor_tensor(
                    msg1[:, sl, :], pt[:],
                    dinv[:, sl, None].to_broadcast([P, 4, D1]),
                    mybir.AluOpType.mult)

            # ---------- y1 = A^T @ msg1 (chunked); v2 = y1 @ W2; msg2 -------
            msg2 = mpool.tile([P, NB, D2], bf16, tag="msg2")
            for ch in range(NCH):
                y1c = psh.tile([P, CH], f32, tag="sh")
                for rb in range(NB):
                    nc.tensor.matmul(
                        y1c[:], msg1[:, rb, :],
                        A_t[ch][:, rb, :],
                        start=(rb == 0), stop=(rb == NB - 1))
                y1q = spool.tile([P, CH], bf16, tag="y1q")
                nc.vector.tensor_copy(y1q[:], y1c[:])
                for j in range(4):
                    nb = ch * 4 + j
                    v2p = psh.tile([P, D2], f32, tag="sh")
                    nc.tensor.matmul(v2p[:], y1q[:, j * P:(j + 1) * P], W2_sb[:],
                                     start=True, stop=True)
                    if with_bias:
                        t = spool.tile([P, D2], f32, tag="tbias")
                        nc.vector.tensor_tensor(
                            t[:], v2p[:], dinv[:, nb:nb + 1].to_broadcast([P, D2]),
                            mybir.AluOpType.mult)
                        nc.vector.tensor_tensor(t[:], t[:], c2_sb[:], mybir.AluOpType.add)
                        nc.vector.tensor_tensor(
                            msg2[:, nb, :], t[:], dinv[:, nb:nb + 1].to_broadcast([P, D2]),
                            mybir.AluOpType.mult)
                    else:
                        nc.vector.tensor_tensor(
                            msg2[:, nb, :], v2p[:], rdeg[:, nb:nb + 1].to_broadcast([P, D2]),
                            mybir.AluOpType.mult)

            # ---------- y2 = A^T @ msg2 (two halves, chunked psum) ----------
            y2h = []
            for half in range(2):
                yh = y2pool.tile([P, N], bf16, tag=f"y2h{half}")
                for ch in range(NCH):
                    y2c = psh.tile([P, CH], f32, tag="sh")
                    for rb in range(NB):
                        nc.tensor.matmul(
                            y2c[:],
                            msg2[:, rb, half * P:(half + 1) * P],
                            A_t[ch][:, rb, :],
                            start=(rb == 0), stop=(rb == NB - 1))
                    nc.vector.tensor_copy(yh[:, ch * CH:(ch + 1) * CH], y2c[:])
                y2h.append(yh)

            # ---------- v3 = y2 @ W3 ; msg3 = rdeg*v3 (+ dinv*c3) ----------
            msg3 = mpool.tile([P, NB, D3], bf16, tag="msg")
            for nb in range(NB):
                v3p = psh.tile([P, D3], f32, tag="sh")
                for half in range(2):
                    nc.tensor.matmul(v3p[:], y2h[half][:, nb * P:(nb + 1) * P],
                                     W3_sb[:, half, :],
                                     start=(half == 0), stop=(half == 1))
                if with_bias:
                    t3 = spool.tile([P, D3], f32, tag="tbias3")
                    nc.vector.tensor_tensor(
                        t3[:], v3p[:], dinv[:, nb:nb + 1].to_broadcast([P, D3]),
                        mybir.AluOpType.mult)
                    nc.vector.tensor_tensor(t3[:], t3[:], c3_sb[:], mybir.AluOpType.add)
                    nc.vector.tensor_tensor(
                        msg3[:, nb, :], t3[:], dinv[:, nb:nb + 1].to_broadcast([P, D3]),
                        mybir.AluOpType.mult)
                else:
                    nc.vector.tensor_tensor(
                        msg3[:, nb, :], v3p[:], rdeg[:, nb:nb + 1].to_broadcast([P, D3]),
                        mybir.AluOpType.mult)

            # ---------- y3 = A^T @ msg3 (A-stationary, grouped) + out -------
            for qg in range(4):
                y3g = psh.tile([P, 4, P], f32, tag="sh")
                for j in range(4):
                    cb = qg * 4 + j
                    for rb in range(NB):
                        nc.tensor.matmul(
                            y3g[:, j, :],
                            A_t[cb // 4][:, rb, (cb % 4) * P:(cb % 4 + 1) * P],
                            msg3[:, rb, :],
                            start=(rb == 0), stop=(rb == NB - 1))
                sl = slice(qg * 4, (qg + 1) * 4)
                og = spool.tile([P, 4, D3], f32, tag="og")
                nc.vector.tensor_tensor(
                    og[:], y3g[:],
                    dinv[:, sl, None].to_broadcast([P, 4, D3]),
                    mybir.AluOpType.mult)
                if with_bias:
                    nc.vector.tensor_tensor(
                        og[:], og[:], b3_sb[:, None, :].to_broadcast([P, 4, D3]),
                        mybir.AluOpType.add)
                nc.sync.dma_start(out_ap[g][:, sl, :], og[:])

    nc.compile()
    return nc


def _get_nc(with_bias):
    key = bool(with_bias)
    if key not in _COMPILED:
        _COMPILED[key] = _build(key)
    return _COMPILED[key]


def kernel(flows, W1, b1, W2, b2, W3, b3, _trace=False):
    from concourse.bass_utils import run_bass_kernel_spmd

    flows = np.asarray(flows, dtype=np.float32)
    W1 = np.asarray(W1, dtype=np.float32)
    W2 = np.asarray(W2, dtype=np.float32)
    W3 = np.asarray(W3, dtype=np.float32)
    b1 = np.asarray(b1, dtype=np.float32)
    b2 = np.asarray(b2, dtype=np.float32)
    b3 = np.asarray(b3, dtype=np.float32)

    with_bias = bool(np.any(b1) or np.any(b2) or np.any(b3))
    nc = _get_nc(with_bias)

    Ab = flows.astype(ml_dtypes.bfloat16)
    Abt = np.ascontiguousarray(Ab.transpose(0, 2, 1))
    W1b = W1.astype(ml_dtypes.bfloat16)
    W2b = W2.astype(ml_dtypes.bfloat16)
    W3b = W3.astype(ml_dtypes.bfloat16)

    in_maps = []
    for c in range(NCORES):
        m = {
            "Ab": Ab[c * GPC:(c + 1) * GPC],
            "Abt": Abt[c * GPC:(c + 1) * GPC],
            "W1b": W1b, "W2b": W2b, "W3b": W3b,
        }
        if with_bias:
            m["c2r"] = np.broadcast_to(b1 @ W2, (P, D2)).copy().astype(np.float32)
            m["c3r"] = np.broadcast_to(b2 @ W3, (P, D3)).copy().astype(np.float32)
            m["b3r"] = np.broadcast_to(b3, (P, D3)).copy().astype(np.float32)
        in_maps.append(m)

    res = run_bass_kernel_spmd(nc, in_maps, core_ids=list(range(NCORES)), trace=_trace)
    out = np.concatenate([res.results[c]["out"] for c in range(NCORES)], axis=0)
    out = np.ascontiguousarray(out.astype(np.float32))
    if _trace:
        return out, res
    return out



# revision 17
# speedup vs baseline: 1.7028x; 1.7028x over previous
"""Trainium2 Bass kernel for nn_Encoder_Flows (3-layer dense GCN message passing).

Math per graph (reference):
    A = flows [N, N];  deg[c] = sum_r A[r, c];  dinv = rsqrt(deg)
    L(x, W, b) = dinv * (A^T @ (dinv * (x @ W))) + b
    out = L(L(L(A, W1, b1), W2, b2), W3, b3)          # [N, 128]

Because every layer is linear, fold the degree normalization into
As = diag(dinv) A diag(dinv) on the host and collapse the right-side weight
chain (P = As^T):
    out = P^3 @ (A @ W123) + (P^2 1) b1W23^T + (P 1) b2W3^T + 1 b3^T
with W123 = W1 W2 W3, all rank-1 bias images host-exact.

Device work per graph is then 4 big [2048 x 2048] @ [2048 x 128] matmuls, run
in fp8(e4m3) with MatmulPerfMode.DoubleRow (2 contraction rows per PE pass).
fp8's error on the node-constant ("mean") component of each intermediate does
not average out in the propagations, so each stage is mean-centered before
quantization: the free-axis mean is measured from PSUM, subtracted in the
quantizing cast, carried forward exactly in f32, and re-injected via
  P (1 xbar^T) = 1 xbar^T + eps xbar^T   (eps = col-sums(As) - 1, host-exact)
where the eps term is a K=128 bf16 matmul against a zero-padded row tile and
the 1*xbar^T term rides the running mean into the final-stage bias add.
The fp8 colsum defect of W123 is likewise fixed by a host-exact rank-1 term
(rowsums(A) x colsum-defect). Data-parallel: 16 graphs / 8 cores.
Output is stored feature-major [128, 2048] and untransposed on the host.
"""

import sys
from contextlib import ExitStack

import numpy as np

for _p in ("/opt/trn_rl_repo", "/opt/pypackages"):
    if _p not in sys.path:
        sys.path.append(_p)

import ml_dtypes

B, N, P = 16, 2048, 128
NB = N // P          # 16 row/col blocks
NCORES = 8
GPC = B // NCORES    # graphs per core
D = 128              # output feature dim
CH = 512             # psum chunk (bank) width
NCH = N // CH        # 4 chunks
NQ = NB // 2         # 8 DoubleRow contraction steps

# fp8 scales (powers of two; validated in numpy sim, >=2x headroom vs absmax)
S_A = 128.0          # raw A (features), entries [0,1)
S_AS = 16384.0       # normalized adjacency As
S_W = 1024.0         # W123 = W1 W2 W3
S_UC = 64.0          # centered u = A @ W123
S_XC = 512.0         # centered x2
S_ZC = 512.0         # centered z2
SIG_U = S_A * S_W
SIG_1 = S_AS * S_UC
SIG_2 = S_AS * S_XC
SIG_3 = S_AS * S_ZC

_COMPILED = {}

FP8NP = ml_dtypes.float8_e4m3
BF16NP = ml_dtypes.bfloat16


def _build(with_bias):
    import concourse.mybir as mybir
    import concourse.tile as tile
    from concourse import bacc
    from concourse.masks import make_identity

    f32 = mybir.dt.float32
    bf16 = mybir.dt.bfloat16
    fp8 = mybir.dt.float8e4
    DR = mybir.MatmulPerfMode.DoubleRow
    X = mybir.AxisListType.X
    MUL = mybir.AluOpType.mult
    SUB = mybir.AluOpType.subtract
    ADD = mybir.AluOpType.add

    nc = bacc.Bacc("TRN2", target_bir_lowering=False)
    Abt_d = nc.declare_dram_parameter("Abt8", [GPC, N, N], fp8, isOutput=False)
    As_d = nc.declare_dram_parameter("As8", [GPC, N, N], fp8, isOutput=False)
    W_d = nc.declare_dram_parameter("Wq8", [N, D], fp8, isOutput=False)
    dS_d = nc.declare_dram_parameter("dSpadT", [P, D], bf16, isOutput=False)
    rA_d = nc.declare_dram_parameter("rArow", [GPC, N], bf16, isOutput=False)
    eps_d = nc.declare_dram_parameter("epsrow", [GPC, N], bf16, isOutput=False)
    if with_bias:
        blhs_d = nc.declare_dram_parameter("blhsT", [P, D], bf16, isOutput=False)
        brow_d = nc.declare_dram_parameter("brows", [GPC, 2, N], bf16, isOutput=False)
        b3_d = nc.declare_dram_parameter("b3col", [P, 1], f32, isOutput=False)
    out_d = nc.declare_dram_parameter("out", [GPC, P, N], f32, isOutput=True)

    with tile.TileContext(nc) as tc, ExitStack() as ctx:
        wpool = ctx.enter_context(tc.tile_pool(name="wpool", bufs=1))
        apool = ctx.enter_context(tc.tile_pool(name="apool", bufs=1))
        spool = ctx.enter_context(tc.tile_pool(name="spool", bufs=2))
        psum = ctx.enter_context(tc.tile_pool(name="psum", bufs=1, space="PSUM"))

        # ---- constants ----
        Wq = wpool.tile([P, NB, D], fp8)
        nc.sync.dma_start(Wq[:], W_d.ap().rearrange("(f p) d -> p f d", p=P))
        dSpadT = wpool.tile([P, D], bf16)
        nc.sync.dma_start(dSpadT[:], dS_d.ap())
        if with_bias:
            blhsT = wpool.tile([P, D], bf16)
            nc.sync.dma_start(blhsT[:], blhs_d.ap())
            b3col = wpool.tile([P, 1], f32)
            nc.sync.dma_start(b3col[:], b3_d.ap())
        iob = wpool.tile([P, P], bf16)
        make_identity(nc, iob[:])

        # zero-padded rank-1 row tiles, one pair per graph (row 0 DMA'd)
        pads = []
        for g in range(GPC):
            rApad = apool.tile([P, N], bf16, tag=f"rApad{g}", name=f"rApad{g}")
            epspad = apool.tile([P, N], bf16, tag=f"epspad{g}", name=f"epspad{g}")
            nc.gpsimd.memset(rApad[:], 0.0)
            nc.gpsimd.memset(epspad[:], 0.0)
            if with_bias:
                bpad = apool.tile([P, N], bf16, tag=f"bpad{g}", name=f"bpad{g}")
                nc.gpsimd.memset(bpad[:], 0.0)
            else:
                bpad = None
            pads.append((rApad, epspad, bpad))

        for g in range(GPC):
            rApad, epspad, bpad = pads[g]
            # ---- input DMAs, in consumption order ----
            abt_t = []
            for t in range(NCH):
                at = spool.tile([P, 4, N], fp8, tag=f"abt{t}", name=f"abt{t}")
                nc.sync.dma_start(
                    at[:],
                    Abt_d.ap()[g][4 * t * P:(4 * t + 4) * P, :]
                    .rearrange("(f p) n -> p f n", p=P))
                abt_t.append(at)
            nc.sync.dma_start(rApad[0:1, :], rA_d.ap()[g:g + 1, :])
            as_t = []
            for t in range(NCH):
                st = spool.tile([P, 4, N], fp8, tag=f"as{t}", name=f"as{t}")
                nc.sync.dma_start(
                    st[:],
                    As_d.ap()[g][4 * t * P:(4 * t + 4) * P, :]
                    .rearrange("(r p) n -> p r n", p=P))
                as_t.append(st)
            nc.sync.dma_start(epspad[0:1, :], eps_d.ap()[g:g + 1, :])
            if with_bias:
                nc.sync.dma_start(bpad[0:2, :], brow_d.ap()[g])

            # small per-stage scratch
            msums = spool.tile([P, 4, 4], f32, tag="msums")   # per-stage chunk sums
            mps = spool.tile([P, 4], f32, tag="mps")          # psum-unit means / stage
            runm = spool.tile([P, 3], f32, tag="runm")        # running means u/x/z
            stagecol = spool.tile([P, P], bf16, tag="stagecol")
            nc.gpsimd.memset(stagecol[:], 0.0)

            def rank1_lhsT(stage, scale_sigma):
                """running mean [P,1] -> bf16 row tile (row0 = xbar*sigma)."""
                nc.vector.tensor_scalar(
                    out=stagecol[:, 0:1], in0=runm[:, stage:stage + 1],
                    scalar1=float(scale_sigma), scalar2=None, op0=MUL)
                xbt_ps = psum.tile([P, P], bf16, tag="xbt", bufs=1, name="xbt_ps")
                nc.tensor.transpose(xbt_ps[:], stagecol[:], iob[:])
                xbT = spool.tile([P, P], bf16, tag=f"xbT{stage}", name="xbT")
                nc.scalar.copy(xbT[:], xbt_ps[:])
                return xbT

            def stage(stage_idx, lhs_tiles, rhs_tiles, rank1s, cast_mult,
                      node_tag, mean_div, prev_stage):
                """One big matmul stage: psum accumulate (+rank-1 terms),
                measure free-axis mean, centering cast to fp8, transpose to
                node-major chunk tiles. Returns node-major chunk tiles."""
                pss = []
                for ch in range(NCH):
                    ps = psum.tile([P, CH], f32, tag="big", bufs=5, name="ps")
                    for q in range(NQ):
                        if len(lhs_tiles) == 1:          # single 16-block tile
                            lhsT = lhs_tiles[0][:, 2 * q:2 * q + 2, :]
                        else:                             # 4-block chunk tiles
                            lt = lhs_tiles[q // 2]
                            lhsT = lt[:, 2 * (q % 2):2 * (q % 2) + 2, :]
                        rt = rhs_tiles[q // 2]
                        rhs = rt[:, 2 * (q % 2):2 * (q % 2) + 2,
                                 ch * CH:(ch + 1) * CH]
                        nc.tensor.matmul(ps[:], lhsT, rhs,
                                         start=(q == 0), stop=False,
                                         perf_mode=DR)
                    for i, (lT, rrow) in enumerate(rank1s):
                        nc.tensor.matmul(
                            ps[:], lT[:], rrow[:, ch * CH:(ch + 1) * CH],
                            start=False, stop=(i == len(rank1s) - 1))
                    nc.vector.reduce_sum(
                        msums[:, stage_idx, ch:ch + 1], ps[:], axis=X)
                    pss.append(ps)
                # combine mean: psum-units and running true-units
                nc.vector.reduce_sum(mps[:, stage_idx:stage_idx + 1],
                                     msums[:, stage_idx, :], axis=X)
                nc.vector.tensor_scalar(
                    out=mps[:, stage_idx:stage_idx + 1],
                    in0=mps[:, stage_idx:stage_idx + 1],
                    scalar1=1.0 / N, scalar2=None, op0=MUL)
                if stage_idx < 3:
                    if prev_stage is None:
                        nc.vector.tensor_scalar(
                            out=runm[:, stage_idx:stage_idx + 1],
                            in0=mps[:, stage_idx:stage_idx + 1],
                            scalar1=1.0 / mean_div, scalar2=None, op0=MUL)
                    else:
                        nc.vector.tensor_scalar(
                            out=runm[:, stage_idx:stage_idx + 1],
                            in0=mps[:, stage_idx:stage_idx + 1],
                            scalar1=1.0 / mean_div,
                            scalar2=runm[:, prev_stage:prev_stage + 1],
                            op0=MUL, op1=ADD)
                if node_tag is None:
                    return pss
                # centering cast + transpose to node-major
                node_tiles = []
                for ch in range(NCH):
                    fq = spool.tile([P, CH], bf16, tag="fq", bufs=4, name="fq")
                    nc.vector.tensor_scalar(
                        out=fq[:], in0=pss[ch][:],
                        scalar1=mps[:, stage_idx:stage_idx + 1],
                        scalar2=float(cast_mult), op0=SUB, op1=MUL)
                    pt = psum.tile([P, 4, P], bf16, tag="pt", bufs=2, name="pt")
                    for j in range(4):
                        nc.tensor.transpose(pt[:, j, :],
                                            fq[:, j * P:(j + 1) * P], iob[:])
                    nt = spool.tile([P, 4, D], fp8, tag=f"{node_tag}{ch}",
                                    name=f"{node_tag}{ch}")
                    nc.scalar.copy(nt[:], pt[:])
                    node_tiles.append(nt)
                return pss, node_tiles

            # ---- stage 0: u = A @ W123 (+ host rank-1 W colsum fix) ----
            _, u_nt = stage(0, [Wq], abt_t, [(dSpadT, rApad)],
                            S_UC / SIG_U, "u", SIG_U, None)
            # ---- stage 1: x2 = P u  (+ eps (x) ubar) ----
            xbT0 = rank1_lhsT(0, SIG_1)
            _, x_nt = stage(1, u_nt, as_t, [(xbT0, epspad)],
                            S_XC / SIG_1, "x", SIG_1, 0)
            # ---- stage 2: z2 = P x2 (+ eps (x) xbar) ----
            xbT1 = rank1_lhsT(1, SIG_2)
            _, z_nt = stage(2, x_nt, as_t, [(xbT1, epspad)],
                            S_ZC / SIG_2, "z", SIG_2, 1)
            # ---- stage 3: out = P z2 (+ eps (x) zbar [+ bias rank-2]) ----
            xbT2 = rank1_lhsT(2, SIG_3)
            r1 = [(xbT2, epspad)]
            if with_bias:
                r1.append((blhsT, bpad))
            pss = stage(3, z_nt, as_t, r1, None, None, None, None)
            # final bias vector: running zbar (+ b3)
            fbias = spool.tile([P, 1], f32, tag="fbias")
            if with_bias:
                nc.vector.tensor_scalar(
                    out=fbias[:], in0=runm[:, 2:3], scalar1=1.0,
                    scalar2=b3col[:], op0=MUL, op1=ADD)
            else:
                nc.vector.tensor_scalar(
                    out=fbias[:], in0=runm[:, 2:3], scalar1=1.0,
                    scalar2=None, op0=MUL)
            for ch in range(NCH):
                osb = spool.tile([P, CH], f32, tag="osb", bufs=4, name="osb")
                nc.vector.tensor_scalar(
                    out=osb[:], in0=pss[ch][:], scalar1=1.0 / SIG_3,
                    scalar2=fbias[:], op0=MUL, op1=ADD)
                nc.sync.dma_start(out_d.ap()[g][:, ch * CH:(ch + 1) * CH],
                                  osb[:])

    nc.compile()
    return nc


def _get_nc(with_bias):
    key = bool(with_bias)
    if key not in _COMPILED:
        _COMPILED[key] = _build(key)
    return _COMPILED[key]


def kernel(flows, W1, b1, W2, b2, W3, b3, _trace=False):
    from concourse.bass_utils import run_bass_kernel_spmd

    flows = np.asarray(flows, dtype=np.float32)
    W1 = np.asarray(W1, dtype=np.float32)
    W2 = np.asarray(W2, dtype=np.float32)
    W3 = np.asarray(W3, dtype=np.float32)
    b1 = np.asarray(b1, dtype=np.float32)
    b2 = np.asarray(b2, dtype=np.float32)
    b3 = np.asarray(b3, dtype=np.float32)

    with_bias = bool(np.any(b1) or np.any(b2) or np.any(b3))
    nc = _get_nc(with_bias)

    # ---- host precompute ----
    W123 = (W1 @ W2 @ W3).astype(np.float32)
    Wq8 = (W123 * S_W).astype(FP8NP)
    dS = (W123 - Wq8.astype(np.float32) / S_W).sum(axis=0)      # [D]
    dSpadT = np.zeros((P, D), dtype=BF16NP)
    dSpadT[0, :] = (dS * (SIG_U / N)).astype(BF16NP)

    deg = flows.sum(axis=1)                                     # [B, N]
    with np.errstate(divide="ignore"):
        dinv = np.where(deg > 0, 1.0 / np.sqrt(deg), 0.0).astype(np.float32)
    As = dinv[:, :, None] * flows * dinv[:, None, :]            # [B, N, N]
    s_col = As.sum(axis=1)                                      # [B, N]
    eps = (s_col - 1.0).astype(BF16NP)
    rA = flows.sum(axis=2).astype(BF16NP)                       # [B, N]

    As8 = (As * S_AS).astype(FP8NP)
    Abt8 = np.ascontiguousarray(
        (flows.transpose(0, 2, 1) * S_A)).astype(FP8NP)

    if with_bias:
        b1W23 = (b1 @ W2 @ W3).astype(np.float32)
        b2W3 = (b2 @ W3).astype(np.float32)
        blhsT = np.zeros((P, D), dtype=BF16NP)
        blhsT[0, :] = b1W23.astype(BF16NP)
        blhsT[1, :] = b2W3.astype(BF16NP)
        Ps = np.einsum("brc,br->bc", As, s_col).astype(np.float32)  # P^2 1
        brows = np.zeros((B, 2, N), dtype=BF16NP)
        brows[:, 0, :] = (Ps * SIG_3).astype(BF16NP)
        brows[:, 1, :] = (s_col * SIG_3).astype(BF16NP)
        b3col = np.ascontiguousarray(b3[:, None]).astype(np.float32)

    in_maps = []
    for c in range(NCORES):
        sl = slice(c * GPC, (c + 1) * GPC)
        m = {
            "Abt8": Abt8[sl],
            "As8": As8[sl],
            "Wq8": Wq8,
            "dSpadT": dSpadT,
            "rArow": rA[sl],
            "epsrow": eps[sl],
        }
        if with_bias:
            m["blhsT"] = blhsT
            m["brows"] = brows[sl]
            m["b3col"] = b3col
        in_maps.append(m)

    res = run_bass_kernel_spmd(nc, in_maps, core_ids=list(range(NCORES)),
                               trace=_trace)
    # out is feature-major [GPC, 128, 2048] per core -> [B, 2048, 128]
    out = np.concatenate(
        [res.results[c]["out"].transpose(0, 2, 1) for c in range(NCORES)],
        axis=0)
    out = np.ascontiguousarray(out.astype(np.float32))
    if _trace:
        return out, res
    return out


# revision 18
# speedup vs baseline: 1.8420x; 1.0817x over previous
"""Trainium2 Bass kernel for nn_Encoder_Flows (3-layer dense GCN message passing).

Math per graph (reference):
    A = flows [N, N];  deg[c] = sum_r A[r, c];  dinv = rsqrt(deg)
    L(x, W, b) = dinv * (A^T @ (dinv * (x @ W))) + b
    out = L(L(L(A, W1, b1), W2, b2), W3, b3)          # [N, 128]

Because every layer is linear, fold the degree normalization into
As = diag(dinv) A diag(dinv) on the host and collapse the right-side weight
chain (P = As^T):
    out = P^3 @ (A @ W123) + (P^2 1) b1W23^T + (P 1) b2W3^T + 1 b3^T
with W123 = W1 W2 W3, all rank-1 bias images host-exact.

Device work per graph is then 4 big [2048 x 2048] @ [2048 x 128] matmuls, run
in fp8(e4m3) with MatmulPerfMode.DoubleRow (2 contraction rows per PE pass).
fp8's error on the node-constant ("mean") component of each intermediate does
not average out in the propagations, so each stage is mean-centered before
quantization: the free-axis mean is measured from PSUM, subtracted in the
quantizing cast, carried forward exactly in f32, and re-injected via
  P (1 xbar^T) = 1 xbar^T + eps xbar^T   (eps = col-sums(As) - 1, host-exact)
where the eps term is a K=128 bf16 matmul against a zero-padded row tile and
the 1*xbar^T term rides the running mean into the final-stage bias add.
The fp8 colsum defect of W123 is likewise fixed by a host-exact rank-1 term
(rowsums(A) x colsum-defect). Data-parallel: 16 graphs / 8 cores.
Output is stored feature-major [128, 2048] and untransposed on the host.
"""

import sys
from contextlib import ExitStack

import numpy as np

for _p in ("/opt/trn_rl_repo", "/opt/pypackages"):
    if _p not in sys.path:
        sys.path.append(_p)

import ml_dtypes

B, N, P = 16, 2048, 128
NB = N // P          # 16 row/col blocks
NCORES = 8
GPC = B // NCORES    # graphs per core
D = 128              # output feature dim
CH = 512             # psum chunk (bank) width
NCH = N // CH        # 4 chunks
NQ = NB // 2         # 8 DoubleRow contraction steps

# fp8 scales (powers of two; validated in numpy sim, >=2x headroom vs absmax)
S_A = 128.0          # raw A (features), entries [0,1)
S_AS = 16384.0       # normalized adjacency As
S_W = 1024.0         # W123 = W1 W2 W3
S_UC = 64.0          # centered u = A @ W123
S_XC = 512.0         # centered x2
S_ZC = 512.0         # centered z2
SIG_U = S_A * S_W
SIG_1 = S_AS * S_UC
SIG_2 = S_AS * S_XC
SIG_3 = S_AS * S_ZC

_COMPILED = {}

FP8NP = ml_dtypes.float8_e4m3
BF16NP = ml_dtypes.bfloat16


def _build(with_bias):
    import concourse.mybir as mybir
    import concourse.tile as tile
    from concourse import bacc
    from concourse.masks import make_identity

    f32 = mybir.dt.float32
    bf16 = mybir.dt.bfloat16
    fp8 = mybir.dt.float8e4
    DR = mybir.MatmulPerfMode.DoubleRow
    X = mybir.AxisListType.X
    MUL = mybir.AluOpType.mult
    SUB = mybir.AluOpType.subtract
    ADD = mybir.AluOpType.add

    nc = bacc.Bacc("TRN2", target_bir_lowering=False)
    Abt_d = nc.declare_dram_parameter("Abt8", [GPC, N, N], fp8, isOutput=False)
    As_d = nc.declare_dram_parameter("As8", [GPC, N, N], fp8, isOutput=False)
    W_d = nc.declare_dram_parameter("Wq8", [N, D], fp8, isOutput=False)
    dS_d = nc.declare_dram_parameter("dSpadT", [P, D], bf16, isOutput=False)
    rA_d = nc.declare_dram_parameter("rArow", [GPC, N], bf16, isOutput=False)
    eps_d = nc.declare_dram_parameter("epsrow", [GPC, N], bf16, isOutput=False)
    if with_bias:
        blhs_d = nc.declare_dram_parameter("blhsT", [P, D], bf16, isOutput=False)
        brow_d = nc.declare_dram_parameter("brows", [GPC, 2, N], bf16, isOutput=False)
        b3_d = nc.declare_dram_parameter("b3col", [P, 1], f32, isOutput=False)
    out_d = nc.declare_dram_parameter("out", [GPC, P, N], f32, isOutput=True)

    with tile.TileContext(nc) as tc, ExitStack() as ctx:
        wpool = ctx.enter_context(tc.tile_pool(name="wpool", bufs=1))
        apool = ctx.enter_context(tc.tile_pool(name="apool", bufs=1))
        spool = ctx.enter_context(tc.tile_pool(name="spool", bufs=2))
        psum = ctx.enter_context(tc.tile_pool(name="psum", bufs=1, space="PSUM"))

        # ---- constants ----
        Wq = wpool.tile([P, NB, D], fp8)
        nc.sync.dma_start(Wq[:], W_d.ap().rearrange("(f p) d -> p f d", p=P))
        dSpadT = wpool.tile([P, D], bf16)
        nc.sync.dma_start(dSpadT[:], dS_d.ap())
        if with_bias:
            blhsT = wpool.tile([P, D], bf16)
            nc.sync.dma_start(blhsT[:], blhs_d.ap())
            b3col = wpool.tile([P, 1], f32)
            nc.sync.dma_start(b3col[:], b3_d.ap())
        iob = wpool.tile([P, P], bf16)
        make_identity(nc, iob[:])

        # ---- per-graph state: pads, DMAs, scratch ----
        G = [dict() for _ in range(GPC)]
        for g in range(GPC):
            gc = G[g]
            gc["rApad"] = apool.tile([P, N], bf16, tag=f"rApad{g}",
                                     name=f"rApad{g}")
            gc["epspad"] = apool.tile([P, N], bf16, tag=f"epspad{g}",
                                      name=f"epspad{g}")
            nc.gpsimd.memset(gc["rApad"][:], 0.0)
            nc.gpsimd.memset(gc["epspad"][:], 0.0)
            if with_bias:
                gc["bpad"] = apool.tile([P, N], bf16, tag=f"bpad{g}",
                                        name=f"bpad{g}")
                nc.gpsimd.memset(gc["bpad"][:], 0.0)
            gc["msums"] = spool.tile([P, 4, 4], f32, tag="msums", name="msums")
            gc["mps"] = spool.tile([P, 4], f32, tag="mps", name="mps")
            gc["runm"] = spool.tile([P, 3], f32, tag="runm", name="runm")
            gc["stagecol"] = spool.tile([P, P], bf16, tag="stagecol",
                                        name="stagecol")
            nc.gpsimd.memset(gc["stagecol"][:], 0.0)

        for g in range(GPC):
            gc = G[g]
            # input DMAs in consumption order; pair-granular A^T tiles so the
            # first DoubleRow matmul starts after ~1 MB has landed
            nc.sync.dma_start(gc["rApad"][0:1, :], rA_d.ap()[g:g + 1, :])
            gc["abt"] = []
            for t in range(NQ):
                at = spool.tile([P, 2, N], fp8, tag=f"abt{t}", name=f"abt{t}")
                nc.sync.dma_start(
                    at[:],
                    Abt_d.ap()[g][2 * t * P:(2 * t + 2) * P, :]
                    .rearrange("(f p) n -> p f n", p=P))
                gc["abt"].append(at)
            nc.sync.dma_start(gc["epspad"][0:1, :], eps_d.ap()[g:g + 1, :])
            gc["as"] = []
            for t in range(NQ):
                st = spool.tile([P, 2, N], fp8, tag=f"as{t}", name=f"as{t}")
                nc.sync.dma_start(
                    st[:],
                    As_d.ap()[g][2 * t * P:(2 * t + 2) * P, :]
                    .rearrange("(r p) n -> p r n", p=P))
                gc["as"].append(st)
            if with_bias:
                nc.sync.dma_start(gc["bpad"][0:2, :], brow_d.ap()[g])

        def rank1_lhsT(gc, stage, scale_sigma):
            """running mean [P,1] -> bf16 row tile (row0 = xbar*sigma)."""
            nc.vector.tensor_scalar(
                out=gc["stagecol"][:, 0:1], in0=gc["runm"][:, stage:stage + 1],
                scalar1=float(scale_sigma), scalar2=None, op0=MUL)
            xbt_ps = psum.tile([P, P], bf16, tag="xbt", bufs=1, name="xbt_ps")
            nc.tensor.transpose(xbt_ps[:], gc["stagecol"][:], iob[:])
            xbT = spool.tile([P, P], bf16, tag=f"xbT{stage}", name="xbT")
            nc.scalar.copy(xbT[:], xbt_ps[:])
            return xbT

        def stage(gc, stage_idx, lhs_tiles, rhs_tiles, rank1s, cast_mult,
                  node_tag, mean_div, prev_stage):
            """One big matmul stage: psum accumulate (+rank-1 terms), measure
            free-axis mean, centering cast, transpose to node-major tiles."""
            msums, mps, runm = gc["msums"], gc["mps"], gc["runm"]
            pss = []
            for ch in range(NCH):
                ps = psum.tile([P, CH], f32, tag="big", bufs=5, name="ps")
                for q in range(NQ):
                    if len(lhs_tiles) == 1:          # single 16-block tile
                        lhsT = lhs_tiles[0][:, 2 * q:2 * q + 2, :]
                    else:                             # 4-block chunk tiles
                        lhsT = lhs_tiles[q // 2][:, 2 * (q % 2):2 * (q % 2) + 2, :]
                    rhs = rhs_tiles[q][:, 0:2, ch * CH:(ch + 1) * CH]
                    nc.tensor.matmul(ps[:], lhsT, rhs,
                                     start=(q == 0), stop=False, perf_mode=DR)
                for i, (lT, rrow) in enumerate(rank1s):
                    nc.tensor.matmul(
                        ps[:], lT[:], rrow[:, ch * CH:(ch + 1) * CH],
                        start=False, stop=(i == len(rank1s) - 1))
                nc.vector.reduce_sum(
                    msums[:, stage_idx, ch:ch + 1], ps[:], axis=X)
                pss.append(ps)
            # combine mean: psum-units and running true-units
            nc.vector.reduce_sum(mps[:, stage_idx:stage_idx + 1],
                                 msums[:, stage_idx, :], axis=X)
            nc.vector.tensor_scalar(
                out=mps[:, stage_idx:stage_idx + 1],
                in0=mps[:, stage_idx:stage_idx + 1],
                scalar1=1.0 / N, scalar2=None, op0=MUL)
            if stage_idx < 3:
                if prev_stage is None:
                    nc.vector.tensor_scalar(
                        out=runm[:, stage_idx:stage_idx + 1],
                        in0=mps[:, stage_idx:stage_idx + 1],
                        scalar1=1.0 / mean_div, scalar2=None, op0=MUL)
                else:
                    nc.vector.tensor_scalar(
                        out=runm[:, stage_idx:stage_idx + 1],
                        in0=mps[:, stage_idx:stage_idx + 1],
                        scalar1=1.0 / mean_div,
                        scalar2=runm[:, prev_stage:prev_stage + 1],
                        op0=MUL, op1=ADD)
            if node_tag is None:
                return pss, None
            # centering cast + transpose to node-major
            node_tiles = []
            for ch in range(NCH):
                fq = spool.tile([P, CH], bf16, tag="fq", bufs=4, name="fq")
                nc.vector.tensor_scalar(
                    out=fq[:], in0=pss[ch][:],
                    scalar1=mps[:, stage_idx:stage_idx + 1],
                    scalar2=float(cast_mult), op0=SUB, op1=MUL)
                pt = psum.tile([P, 4, P], bf16, tag="pt", bufs=2, name="pt")
                for j in range(4):
                    nc.tensor.transpose(pt[:, j, :],
                                        fq[:, j * P:(j + 1) * P], iob[:])
                nt = spool.tile([P, 4, D], fp8, tag=f"{node_tag}{ch}",
                                name=f"{node_tag}{ch}")
                nc.scalar.copy(nt[:], pt[:])
                node_tiles.append(nt)
            return pss, node_tiles

        # ---- stage-major over graphs: keeps the PE busy across the
        # per-graph mean/cast/transpose chains (and the HAM clock warm) ----
        for g in range(GPC):
            gc = G[g]
            _, gc["u_nt"] = stage(gc, 0, [Wq], gc["abt"],
                                  [(dSpadT, gc["rApad"])],
                                  S_UC / SIG_U, "u", SIG_U, None)
        for g in range(GPC):
            gc = G[g]
            xbT0 = rank1_lhsT(gc, 0, SIG_1)
            _, gc["x_nt"] = stage(gc, 1, gc["u_nt"], gc["as"],
                                  [(xbT0, gc["epspad"])],
                                  S_XC / SIG_1, "x", SIG_1, 0)
        for g in range(GPC):
            gc = G[g]
            xbT1 = rank1_lhsT(gc, 1, SIG_2)
            _, gc["z_nt"] = stage(gc, 2, gc["x_nt"], gc["as"],
                                  [(xbT1, gc["epspad"])],
                                  S_ZC / SIG_2, "z", SIG_2, 1)
        for g in range(GPC):
            gc = G[g]
            xbT2 = rank1_lhsT(gc, 2, SIG_3)
            r1 = [(xbT2, gc["epspad"])]
            if with_bias:
                r1.append((blhsT, gc["bpad"]))
            pss, _ = stage(gc, 3, gc["z_nt"], gc["as"], r1,
                           None, None, None, None)
            # final bias vector: running zbar (+ b3)
            fbias = spool.tile([P, 1], f32, tag="fbias", name="fbias")
            if with_bias:
                nc.vector.tensor_scalar(
                    out=fbias[:], in0=gc["runm"][:, 2:3], scalar1=1.0,
                    scalar2=b3col[:], op0=MUL, op1=ADD)
            else:
                nc.vector.tensor_scalar(
                    out=fbias[:], in0=gc["runm"][:, 2:3], scalar1=1.0,
                    scalar2=None, op0=MUL)
            for ch in range(NCH):
                osb = spool.tile([P, CH], f32, tag="osb", bufs=4, name="osb")
                nc.vector.tensor_scalar(
                    out=osb[:], in0=pss[ch][:], scalar1=1.0 / SIG_3,
                    scalar2=fbias[:], op0=MUL, op1=ADD)
                nc.sync.dma_start(out_d.ap()[g][:, ch * CH:(ch + 1) * CH],
                                  osb[:])

    nc.compile()
    return nc


def _get_nc(with_bias):
    key = bool(with_bias)
    if key not in _COMPILED:
        _COMPILED[key] = _build(key)
    return _COMPILED[key]


def kernel(flows, W1, b1, W2, b2, W3, b3, _trace=False):
    from concourse.bass_utils import run_bass_kernel_spmd

    flows = np.asarray(flows, dtype=np.float32)
    W1 = np.asarray(W1, dtype=np.float32)
    W2 = np.asarray(W2, dtype=np.float32)
    W3 = np.asarray(W3, dtype=np.float32)
    b1 = np.asarray(b1, dtype=np.float32)
    b2 = np.asarray(b2, dtype=np.float32)
    b3 = np.asarray(b3, dtype=np.float32)

    with_bias = bool(np.any(b1) or np.any(b2) or np.any(b3))
    nc = _get_nc(with_bias)

    # ---- host precompute ----
    W123 = (W1 @ W2 @ W3).astype(np.float32)
    Wq8 = (W123 * S_W).astype(FP8NP)
    dS = (W123 - Wq8.astype(np.float32) / S_W).sum(axis=0)      # [D]
    dSpadT = np.zeros((P, D), dtype=BF16NP)
    dSpadT[0, :] = (dS * (SIG_U / N)).astype(BF16NP)

    deg = flows.sum(axis=1)                                     # [B, N]
    with np.errstate(divide="ignore"):
        dinv = np.where(deg > 0, 1.0 / np.sqrt(deg), 0.0).astype(np.float32)
    As = dinv[:, :, None] * flows * dinv[:, None, :]            # [B, N, N]
    s_col = As.sum(axis=1)                                      # [B, N]
    eps = (s_col - 1.0).astype(BF16NP)
    rA = flows.sum(axis=2).astype(BF16NP)                       # [B, N]

    As8 = (As * S_AS).astype(FP8NP)
    Abt8 = np.ascontiguousarray(
        (flows.transpose(0, 2, 1) * S_A)).astype(FP8NP)

    if with_bias:
        b1W23 = (b1 @ W2 @ W3).astype(np.float32)
        b2W3 = (b2 @ W3).astype(np.float32)
        blhsT = np.zeros((P, D), dtype=BF16NP)
        blhsT[0, :] = b1W23.astype(BF16NP)
        blhsT[1, :] = b2W3.astype(BF16NP)
        Ps = np.einsum("brc,br->bc", As, s_col).astype(np.float32)  # P^2 1
        brows = np.zeros((B, 2, N), dtype=BF16NP)
        brows[:, 0, :] = (Ps * SIG_3).astype(BF16NP)
        brows[:, 1, :] = (s_col * SIG_3).astype(BF16NP)
        b3col = np.ascontiguousarray(b3[:, None]).astype(np.float32)

    in_maps = []
    for c in range(NCORES):
        sl = slice(c * GPC, (c + 1) * GPC)
        m = {
            "Abt8": Abt8[sl],
            "As8": As8[sl],
            "Wq8": Wq8,
            "dSpadT": dSpadT,
            "rArow": rA[sl],
            "epsrow": eps[sl],
        }
        if with_bias:
            m["blhsT"] = blhsT
            m["brows"] = brows[sl]
            m["b3col"] = b3col
        in_maps.append(m)

    res = run_bass_kernel_spmd(nc, in_maps, core_ids=list(range(NCORES)),
                               trace=_trace)
    # out is feature-major [GPC, 128, 2048] per core -> [B, 2048, 128]
    out = np.concatenate(
        [res.results[c]["out"].transpose(0, 2, 1) for c in range(NCORES)],
        axis=0)
    out = np.ascontiguousarray(out.astype(np.float32))
    if _trace:
        return out, res
    return out
